# revision 20
# baseline (speedup 1.0000x reference)
"""Trainium2 Bass kernel for nn_BranchingQNetwork (12-branch dueling Q-MLP).

Strategy: data-parallel over batch (8 cores x 1024 rows). Per core, all 12
branch MLPs run as feature-major GEMM chains (weights stationary, activations
streaming) in bf16, k-outer single-pass accumulation in PSUM banks with
weights streamed through a small SBUF window. Layer-1 is zero-padded to
K=128 (pad rows of the px tile carry arbitrary finite x data times zero
weights) so every matmul has an identical 128-row shape and the PE never
reconfigures. Relu drains are whole-tile ops alternating between the scalar
and vector engines, emitted inline right after each accumulation group stops
so PSUM banks recycle early. The dueling head (v + a - mean(a)) is linear
and folded into a single [512, 11] weight on the host; it runs with Wq
stationary (11-column LDWEIGHTS, N=512 streaming) and the [11, batch] output
is transposed on the host. DMA queues: w2 on sync (prefetched 4 tiles ahead
into the next iteration), w3 + input/branch loads + output on gpsimd.
"""
import sys

sys.path.insert(0, "/opt/trn_rl_repo")

import numpy as np
import ml_dtypes

# problem dims (hardcoded per harness contract)
B = 8192
OBS = 249
NB = 12
NA = 11
NODE = 45
GRP = 17
D0 = 62
D1 = 2048
D2 = 1024
D3 = 512

NCORES = 8
LB = B // NCORES     # local batch per core
BT = 512             # batch tile
NBT = LB // BT
M1 = D1 // 128       # 16 output tiles of layer 1
K2 = D1 // 128       # 16 contraction tiles of layer 2
M2 = D2 // 128       # 8
K3 = D2 // 128       # 8
M3 = D3 // 128       # 4
KH = D3 // 128       # 4
PREF = 4             # w2 tiles prefetched during previous iteration

BF16 = ml_dtypes.bfloat16

_NC_CACHE = {}
LAST_RESULT = None


def _build_nc():
    if "nc" in _NC_CACHE:
        return _NC_CACHE["nc"]
    from concourse import bacc
    import concourse.mybir as mybir
    import concourse.tile as tile

    f32 = mybir.dt.float32
    bf16 = mybir.dt.bfloat16
    Relu = mybir.ActivationFunctionType.Relu
    Identity = mybir.ActivationFunctionType.Identity
    ADD = mybir.AluOpType.add
    MAX = mybir.AluOpType.max

    nc = bacc.Bacc("TRN2")

    xT_d = nc.declare_dram_parameter("xT", [OBS, LB], bf16, isOutput=False)
    W1_d = nc.declare_dram_parameter("W1p", [NB, 128, D1], bf16, isOutput=False)
    W2_d = nc.declare_dram_parameter("W2p", [NB, K2, 128, D2], bf16, isOutput=False)
    W3_d = nc.declare_dram_parameter("W3p", [NB, K3, 128, D3], bf16, isOutput=False)
    Wq_d = nc.declare_dram_parameter("Wqp", [NB, KH, 128, 128], bf16, isOutput=False)
    b_d = nc.declare_dram_parameter("bp", [NB, 128, M1 + M2 + M3], f32, isOutput=False)
    bq_d = nc.declare_dram_parameter("bqp", [NB, NA, 1], f32, isOutput=False)
    out_d = nc.declare_dram_parameter("out", [NB, NA, LB], f32, isOutput=True)

    with tile.TileContext(nc) as tc:
        with (
            tc.tile_pool(name="wp1", bufs=2) as wp1,
            tc.tile_pool(name="wp2", bufs=12) as wp2,
            tc.tile_pool(name="wp3", bufs=8) as wp3,
            tc.tile_pool(name="wpq", bufs=2) as wpq,
            tc.tile_pool(name="bbp", bufs=2) as bbp,
            tc.tile_pool(name="pxp", bufs=3) as pxp,
            tc.tile_pool(name="actp", bufs=1) as actp,
            tc.tile_pool(name="osp", bufs=3) as osp,
            tc.tile_pool(name="psp", bufs=8, space="PSUM") as psp,
        ):
            h1 = actp.tile([128, K2, BT], bf16, tag="h1")
            h2 = actp.tile([128, K3, BT], bf16, tag="h2")
            h3 = actp.tile([128, KH, BT], bf16, tag="h3")

            iters = [(br, bt) for br in range(NB) for bt in range(NBT)]
            loaded = {}
            pxs = {}
            w2ts = {}
            w2_issued = {}

            def load_branch(br, eng):
                w1t = wp1.tile([128, D1], bf16, tag="w1", name=f"w1_{br}")
                eng.dma_start(w1t[:], W1_d[br])
                wqt = wpq.tile([128, KH, 128], bf16, tag="wq", name=f"wq_{br}")
                eng.dma_start(wqt[:], Wq_d[br].rearrange("k p a -> p k a"))
                btile = bbp.tile([128, M1 + M2 + M3], f32, tag="b", name=f"b_{br}")
                eng.dma_start(btile[:], b_d[br])
                bqt = bbp.tile([NA, 1], f32, tag="bq", name=f"bq_{br}")
                eng.dma_start(bqt[:], bq_d[br])
                loaded[br] = (w1t, wqt, btile, bqt)

            def load_px(idx, eng):
                br, bt = iters[idx]
                bsl = slice(bt * BT, (bt + 1) * BT)
                g0 = NODE + GRP * br
                px = pxp.tile([128, BT], bf16, tag="px", name=f"px_{idx}")
                eng.dma_start(px[0:NODE, :], xT_d[0:NODE, bsl])
                eng.dma_start(px[NODE:D0, :], xT_d[g0:g0 + GRP, bsl])
                # rows 62-127 multiply zero weight rows; fill with finite x
                # data (never NaN) instead of memset to stay on one queue
                eng.dma_start(px[D0:128, :], xT_d[0:128 - D0, bsl])
                pxs[idx] = px

            def make_w2(idx):
                if idx in w2ts:
                    return
                w2ts[idx] = [
                    wp2.tile([128, D2], bf16, tag="w2", name=f"w2_{idx}_{k}")
                    for k in range(K2)
                ]
                w2_issued[idx] = set()

            def issue_w2(idx, k):
                if k >= K2 or k in w2_issued[idx]:
                    return
                w2_issued[idx].add(k)
                br = iters[idx][0]
                nc.sync.dma_start(w2ts[idx][k][:], W2_d[br, k])

            def drain(dst, ps, bias, j):
                if j % 2 == 0:
                    nc.scalar.activation(dst, ps, Relu, bias=bias, scale=1.0)
                else:
                    nc.vector.tensor_scalar(dst, ps, bias, 0.0, ADD, MAX)

            def emit_L1_mm(idx, m):
                br, _ = iters[idx]
                w1t, _, btile, _ = loaded[br]
                ps = psp.tile([128, BT], f32, tag="ps", name=f"l1ps_{idx}_{m}")
                nc.tensor.matmul(
                    ps[:], w1t[:, m * 128:(m + 1) * 128], pxs[idx][:],
                    start=True, stop=True,
                )
                drain(h1[:, m, :], ps[:], btile[:, m:m + 1], m)

            # prologue: w1 + px + biases first (layer-1 critical path), then
            # the first two w2 tiles (not needed until ~3.5us in)
            w1t0 = wp1.tile([128, D1], bf16, tag="w1", name="w1_0")
            nc.sync.dma_start(w1t0[:], W1_d[0])
            load_px(0, nc.sync)
            btile0 = bbp.tile([128, M1 + M2 + M3], f32, tag="b", name="b_0")
            nc.sync.dma_start(btile0[:], b_d[0])
            make_w2(0)
            issue_w2(0, 0)
            issue_w2(0, 1)
            wqt0 = wpq.tile([128, KH, 128], bf16, tag="wq", name="wq_0")
            nc.sync.dma_start(wqt0[:], Wq_d[0].rearrange("k p a -> p k a"))
            bqt0 = bbp.tile([NA, 1], f32, tag="bq", name="bq_0")
            nc.sync.dma_start(bqt0[:], bq_d[0])
            loaded[0] = (w1t0, wqt0, btile0, bqt0)
            for m in range(M1):
                emit_L1_mm(0, m)

            for idx, (br, bt) in enumerate(iters):
                w1t, wqt, btile, bqt = loaded[br]
                nxt = idx + 1 if idx + 1 < len(iters) else None

                # this iteration's w3 tiles stream on the gpsimd queue
                w3list = []
                for k in range(K3):
                    w3t = wp3.tile([128, D3], bf16, tag="w3", name=f"w3_{idx}_{k}")
                    nc.gpsimd.dma_start(w3t[:], W3_d[br, k])
                    w3list.append(w3t)
                if nxt is not None:
                    nbr = iters[nxt][0]
                    if nbr not in loaded:
                        load_branch(nbr, nc.gpsimd)
                    load_px(nxt, nc.gpsimd)
                    make_w2(nxt)

                # ---- L2: [2048 -> 1024], k-outer, 8 psum banks ----
                ps2 = [psp.tile([128, BT], f32, tag="ps", name=f"ps2_{idx}_{m}")
                       for m in range(M2)]
                for k in range(K2):
                    issue_w2(idx, k)
                    issue_w2(idx, k + 2)
                    w2t = w2ts[idx][k]
                    for m in range(M2):
                        nc.tensor.matmul(
                            ps2[m][:], w2t[:, m * 128:(m + 1) * 128],
                            h1[:, k, :],
                            start=(k == 0), stop=(k == K2 - 1),
                        )
                        if k == K2 - 1:
                            drain(h2[:, m, :], ps2[m][:],
                                  btile[:, M1 + m:M1 + m + 1], m)

                # ---- L3 [1024 -> 512] interleaved with next iteration's L1 ----
                ps3 = [psp.tile([128, BT], f32, tag="ps", name=f"ps3_{idx}_{m}")
                       for m in range(M3)]
                for k in range(K3):
                    # in the final k block the L1 pair goes first so the h3
                    # drains finish before the head matmuls need them
                    if k == K3 - 1 and nxt is not None:
                        emit_L1_mm(nxt, 2 * k)
                        emit_L1_mm(nxt, 2 * k + 1)
                    for m in range(M3):
                        nc.tensor.matmul(
                            ps3[m][:], w3list[k][:, m * 128:(m + 1) * 128],
                            h2[:, k, :],
                            start=(k == 0), stop=(k == K3 - 1),
                        )
                        if k == K3 - 1:
                            drain(h3[:, m, :], ps3[m][:],
                                  btile[:, M1 + M2 + m:M1 + M2 + m + 1], m)
                    if k < K3 - 1 and nxt is not None:
                        emit_L1_mm(nxt, 2 * k)
                        emit_L1_mm(nxt, 2 * k + 1)
                    if nxt is not None and k < PREF:
                        issue_w2(nxt, k)

                # ---- head: q = h3 @ Wq + bq, Wq stationary, [11, BT] out.
                # Two psum banks alternate so consecutive head matmuls never
                # hit the same bank (same-bank accumulation stalls ~95ns).
                # Ring padded to 32 allocs/iter for a stable rotation, with
                # psh_a/psh_b last so only the late ps2[6]/ps2[7] of the next
                # iteration inherit their banks.
                bsl = slice(bt * BT, (bt + 1) * BT)
                for p in range(2):
                    psp.tile([128, BT], f32, tag="ps", name=f"pad_{idx}_{p}")
                psh_a = psp.tile([128, BT], f32, tag="ps", name=f"psha_{idx}")
                psh_b = psp.tile([128, BT], f32, tag="ps", name=f"pshb_{idx}")
                for k in range(KH):
                    ps = psh_a if k % 2 == 0 else psh_b
                    nc.tensor.matmul(
                        ps[:], wqt[:, k, :], h3[:, k, :],
                        start=(k < 2), stop=(k >= 2),
                    )
                ost = osp.tile([NA, BT], f32, tag="os", name=f"ost_{idx}")
                nc.scalar.activation(ost[:], psh_a[0:NA, :], Identity,
                                     bias=bqt[:], scale=1.0)
                nc.vector.tensor_add(ost[:], ost[:], psh_b[0:NA, :])
                nc.gpsimd.dma_start(out_d[br, :, bsl], ost[:])

    nc.compile()
    _NC_CACHE["nc"] = nc
    return nc


def _pack_weights(W1, b1, W2, b2, W3, b3, Wv, bv, Wa, ba):
    f = np.float32
    W1p = np.zeros((NB, 128, D1), np.float32)
    W1p[:, :D0, :] = W1
    W1p = W1p.astype(BF16)                                       # [12, 128, 2048]
    W2p = np.ascontiguousarray(W2.reshape(NB, K2, 128, D2)).astype(BF16)
    W3p = np.ascontiguousarray(W3.reshape(NB, K3, 128, D3)).astype(BF16)
    # fold dueling head: q = h @ (Wv + Wa - mean(Wa)) + (bv + ba - mean(ba))
    Wq = Wv + Wa - Wa.mean(axis=2, keepdims=True)                # [12, 512, 11]
    bq = bv + ba - ba.mean(axis=1, keepdims=True)                # [12, 11]
    Wqp = np.zeros((NB, KH, 128, 128), np.float32)
    Wqp[:, :, :, :NA] = Wq.reshape(NB, KH, 128, NA)
    Wqp = Wqp.astype(BF16)
    bp = np.concatenate(
        [
            b1.reshape(NB, M1, 128).transpose(0, 2, 1),
            b2.reshape(NB, M2, 128).transpose(0, 2, 1),
            b3.reshape(NB, M3, 128).transpose(0, 2, 1),
        ],
        axis=2,
    ).astype(f)                                                  # [12, 128, 28]
    bqp = np.ascontiguousarray(bq[:, :, None], f)                # [12, 11, 1]
    return W1p, W2p, W3p, Wqp, bp, bqp


def kernel(x, W1, b1, W2, b2, W3, b3, Wv, bv, Wa, ba):
    global LAST_RESULT
    from concourse.bass_utils import run_bass_kernel_spmd

    x = np.asarray(x, np.float32)
    args = [np.asarray(a, np.float32) for a in (W1, b1, W2, b2, W3, b3, Wv, bv, Wa, ba)]
    W1p, W2p, W3p, Wqp, bp, bqp = _pack_weights(*args)

    nc = _build_nc()
    in_maps = []
    for c in range(NCORES):
        xT = np.ascontiguousarray(x[c * LB:(c + 1) * LB].T).astype(BF16)  # [249, 1024]
        in_maps.append({
            "xT": xT,
            "W1p": W1p, "W2p": W2p, "W3p": W3p, "Wqp": Wqp,
            "bp": bp, "bqp": bqp,
        })

    res = run_bass_kernel_spmd(nc, in_maps, list(range(NCORES)))
    LAST_RESULT = res

    out = np.empty((NB, B, NA), np.float32)
    for c in range(NCORES):
        out[:, c * LB:(c + 1) * LB, :] = res.results[c]["out"].transpose(0, 2, 1)
    return out


# revision 21
# speedup vs baseline: 1.0099x; 1.0099x over previous
"""Trainium2 Bass kernel for nn_BranchingQNetwork (12-branch dueling Q-MLP).

Strategy: data-parallel over batch (8 cores x 1024 rows). Per core, all 12
branch MLPs run as feature-major GEMM chains (weights stationary, activations
streaming) in bf16, k-outer single-pass accumulation in PSUM banks with
weights streamed through a small SBUF window. Layer-1 is zero-padded to
K=128 (pad rows of the px tile carry arbitrary finite x data times zero
weights) so every matmul has an identical 128-row shape and the PE never
reconfigures. Relu drains are whole-tile ops alternating between the scalar
and vector engines, emitted inline right after each accumulation group stops
so PSUM banks recycle early. The dueling head (v + a - mean(a)) is linear
and folded into a single [512, 11] weight on the host; it runs with Wq
stationary (11-column LDWEIGHTS, N=512 streaming) and the [11, batch] output
is transposed on the host. DMA queues: w2 on sync (prefetched 4 tiles ahead
into the next iteration), w3 + input/branch loads + output on gpsimd.
"""
import sys

sys.path.insert(0, "/opt/trn_rl_repo")

import numpy as np
import ml_dtypes

# problem dims (hardcoded per harness contract)
B = 8192
OBS = 249
NB = 12
NA = 11
NODE = 45
GRP = 17
D0 = 62
D1 = 2048
D2 = 1024
D3 = 512

NCORES = 8
LB = B // NCORES     # local batch per core
BT = 512             # batch tile
NBT = LB // BT
M1 = D1 // 128       # 16 output tiles of layer 1
K2 = D1 // 128       # 16 contraction tiles of layer 2
M2 = D2 // 128       # 8
K3 = D2 // 128       # 8
M3 = D3 // 128       # 4
KH = D3 // 128       # 4
PREF = 4             # w2 tiles prefetched during previous iteration

BF16 = ml_dtypes.bfloat16

_NC_CACHE = {}
LAST_RESULT = None


def _build_nc():
    if "nc" in _NC_CACHE:
        return _NC_CACHE["nc"]
    from concourse import bacc
    import concourse.mybir as mybir
    import concourse.tile as tile

    f32 = mybir.dt.float32
    bf16 = mybir.dt.bfloat16
    Relu = mybir.ActivationFunctionType.Relu
    Identity = mybir.ActivationFunctionType.Identity
    ADD = mybir.AluOpType.add
    MAX = mybir.AluOpType.max

    nc = bacc.Bacc("TRN2")

    xT_d = nc.declare_dram_parameter("xT", [OBS, LB], bf16, isOutput=False)
    W1_d = nc.declare_dram_parameter("W1p", [NB, 128, D1], bf16, isOutput=False)
    W2_d = nc.declare_dram_parameter("W2p", [NB, K2, 128, D2], bf16, isOutput=False)
    W3_d = nc.declare_dram_parameter("W3p", [NB, K3, 128, D3], bf16, isOutput=False)
    Wq_d = nc.declare_dram_parameter("Wqp", [NB, KH, 128, 128], bf16, isOutput=False)
    b_d = nc.declare_dram_parameter("bp", [NB, 128, M1 + M2 + M3], f32, isOutput=False)
    bq_d = nc.declare_dram_parameter("bqp", [NB, NA, 1], f32, isOutput=False)
    out_d = nc.declare_dram_parameter("out", [NB, NA, LB], f32, isOutput=True)

    with tile.TileContext(nc) as tc:
        with (
            tc.tile_pool(name="wp1", bufs=2) as wp1,
            tc.tile_pool(name="wp2", bufs=12) as wp2,
            tc.tile_pool(name="wp3", bufs=8) as wp3,
            tc.tile_pool(name="wpq", bufs=2) as wpq,
            tc.tile_pool(name="bbp", bufs=2) as bbp,
            tc.tile_pool(name="pxp", bufs=3) as pxp,
            tc.tile_pool(name="actp", bufs=1) as actp,
            tc.tile_pool(name="osp", bufs=3) as osp,
            tc.tile_pool(name="psp", bufs=8, space="PSUM") as psp,
        ):
            h1 = actp.tile([128, K2, BT], bf16, tag="h1")
            h2 = actp.tile([128, K3, BT], bf16, tag="h2")
            h3 = actp.tile([128, KH, BT], bf16, tag="h3")

            iters = [(br, bt) for br in range(NB) for bt in range(NBT)]
            loaded = {}
            pxs = {}
            w2ts = {}
            w2_issued = {}

            def load_branch(br, eng):
                w1t = wp1.tile([128, D1], bf16, tag="w1", name=f"w1_{br}")
                eng.dma_start(w1t[:], W1_d[br])
                wqt = wpq.tile([128, KH, 128], bf16, tag="wq", name=f"wq_{br}")
                eng.dma_start(wqt[:], Wq_d[br].rearrange("k p a -> p k a"))
                btile = bbp.tile([128, M1 + M2 + M3], f32, tag="b", name=f"b_{br}")
                eng.dma_start(btile[:], b_d[br])
                bqt = bbp.tile([NA, 1], f32, tag="bq", name=f"bq_{br}")
                eng.dma_start(bqt[:], bq_d[br])
                loaded[br] = (w1t, wqt, btile, bqt)

            def load_px(idx, eng):
                br, bt = iters[idx]
                bsl = slice(bt * BT, (bt + 1) * BT)
                g0 = NODE + GRP * br
                px = pxp.tile([128, BT], bf16, tag="px", name=f"px_{idx}")
                eng.dma_start(px[0:NODE, :], xT_d[0:NODE, bsl])
                eng.dma_start(px[NODE:D0, :], xT_d[g0:g0 + GRP, bsl])
                # rows 62-127 multiply zero weight rows; fill with finite x
                # data (never NaN) instead of memset to stay on one queue
                eng.dma_start(px[D0:128, :], xT_d[0:128 - D0, bsl])
                pxs[idx] = px

            def make_w2(idx):
                if idx in w2ts:
                    return
                w2ts[idx] = [
                    wp2.tile([128, D2], bf16, tag="w2", name=f"w2_{idx}_{k}")
                    for k in range(K2)
                ]
                w2_issued[idx] = set()

            def issue_w2(idx, k):
                if k >= K2 or k in w2_issued[idx]:
                    return
                w2_issued[idx].add(k)
                br = iters[idx][0]
                nc.sync.dma_start(w2ts[idx][k][:], W2_d[br, k])

            def drain(dst, ps, bias, j):
                if j % 2 == 0:
                    nc.scalar.activation(dst, ps, Relu, bias=bias, scale=1.0)
                else:
                    nc.vector.tensor_scalar(dst, ps, bias, 0.0, ADD, MAX)

            def emit_L1_mm(idx, m):
                br, _ = iters[idx]
                w1t, _, btile, _ = loaded[br]
                ps = psp.tile([128, BT], f32, tag="ps", name=f"l1ps_{idx}_{m}")
                nc.tensor.matmul(
                    ps[:], w1t[:, m * 128:(m + 1) * 128], pxs[idx][:],
                    start=True, stop=True,
                )
                drain(h1[:, m, :], ps[:], btile[:, m:m + 1], m)

            # prologue: w2[0:2] first so L2 never waits, then branch + px
            make_w2(0)
            issue_w2(0, 0)
            issue_w2(0, 1)
            load_branch(0, nc.sync)
            load_px(0, nc.sync)
            for m in range(M1):
                emit_L1_mm(0, m)

            for idx, (br, bt) in enumerate(iters):
                w1t, wqt, btile, bqt = loaded[br]
                nxt = idx + 1 if idx + 1 < len(iters) else None

                # this iteration's w3 tiles stream on the gpsimd queue
                w3list = []
                for k in range(K3):
                    w3t = wp3.tile([128, D3], bf16, tag="w3", name=f"w3_{idx}_{k}")
                    nc.gpsimd.dma_start(w3t[:], W3_d[br, k])
                    w3list.append(w3t)
                if nxt is not None:
                    nbr = iters[nxt][0]
                    if nbr not in loaded:
                        load_branch(nbr, nc.gpsimd)
                    load_px(nxt, nc.gpsimd)
                    make_w2(nxt)

                # ---- L2: [2048 -> 1024], k-outer, 8 psum banks ----
                ps2 = [psp.tile([128, BT], f32, tag="ps", name=f"ps2_{idx}_{m}")
                       for m in range(M2)]
                for k in range(K2):
                    issue_w2(idx, k)
                    issue_w2(idx, k + 2)
                    w2t = w2ts[idx][k]
                    for m in range(M2):
                        nc.tensor.matmul(
                            ps2[m][:], w2t[:, m * 128:(m + 1) * 128],
                            h1[:, k, :],
                            start=(k == 0), stop=(k == K2 - 1),
                        )
                        if k == K2 - 1:
                            drain(h2[:, m, :], ps2[m][:],
                                  btile[:, M1 + m:M1 + m + 1], m)

                # ---- L3 [1024 -> 512] interleaved with next iteration's L1 ----
                ps3 = [psp.tile([128, BT], f32, tag="ps", name=f"ps3_{idx}_{m}")
                       for m in range(M3)]
                for k in range(K3):
                    for m in range(M3):
                        nc.tensor.matmul(
                            ps3[m][:], w3list[k][:, m * 128:(m + 1) * 128],
                            h2[:, k, :],
                            start=(k == 0), stop=(k == K3 - 1),
                        )
                        if k == K3 - 1:
                            drain(h3[:, m, :], ps3[m][:],
                                  btile[:, M1 + M2 + m:M1 + M2 + m + 1], m)
                    if nxt is not None:
                        emit_L1_mm(nxt, 2 * k)
                        emit_L1_mm(nxt, 2 * k + 1)
                        if k < PREF:
                            issue_w2(nxt, k)

                # ---- head: q = h3 @ Wq + bq, Wq stationary, [11, BT] out.
                # Two psum banks alternate so consecutive head matmuls never
                # hit the same bank (same-bank accumulation stalls ~95ns).
                # Ring padded to 32 allocs/iter for a stable rotation, with
                # psh_a/psh_b last so only the late ps2[6]/ps2[7] of the next
                # iteration inherit their banks.
                bsl = slice(bt * BT, (bt + 1) * BT)
                for p in range(2):
                    psp.tile([128, BT], f32, tag="ps", name=f"pad_{idx}_{p}")
                psh_a = psp.tile([128, BT], f32, tag="ps", name=f"psha_{idx}")
                psh_b = psp.tile([128, BT], f32, tag="ps", name=f"pshb_{idx}")
                for k in range(KH):
                    ps = psh_a if k % 2 == 0 else psh_b
                    nc.tensor.matmul(
                        ps[:], wqt[:, k, :], h3[:, k, :],
                        start=(k < 2), stop=(k >= 2),
                    )
                ost = osp.tile([NA, BT], f32, tag="os", name=f"ost_{idx}")
                nc.scalar.activation(ost[:], psh_a[0:NA, :], Identity,
                                     bias=bqt[:], scale=1.0)
                nc.vector.tensor_add(ost[:], ost[:], psh_b[0:NA, :])
                nc.gpsimd.dma_start(out_d[br, :, bsl], ost[:])

    nc.compile()
    _NC_CACHE["nc"] = nc
    return nc


def _pack_weights(W1, b1, W2, b2, W3, b3, Wv, bv, Wa, ba):
    f = np.float32
    W1p = np.zeros((NB, 128, D1), np.float32)
    W1p[:, :D0, :] = W1
    W1p = W1p.astype(BF16)                                       # [12, 128, 2048]
    W2p = np.ascontiguousarray(W2.reshape(NB, K2, 128, D2)).astype(BF16)
    W3p = np.ascontiguousarray(W3.reshape(NB, K3, 128, D3)).astype(BF16)
    # fold dueling head: q = h @ (Wv + Wa - mean(Wa)) + (bv + ba - mean(ba))
    Wq = Wv + Wa - Wa.mean(axis=2, keepdims=True)                # [12, 512, 11]
    bq = bv + ba - ba.mean(axis=1, keepdims=True)                # [12, 11]
    Wqp = np.zeros((NB, KH, 128, 128), np.float32)
    Wqp[:, :, :, :NA] = Wq.reshape(NB, KH, 128, NA)
    Wqp = Wqp.astype(BF16)
    bp = np.concatenate(
        [
            b1.reshape(NB, M1, 128).transpose(0, 2, 1),
            b2.reshape(NB, M2, 128).transpose(0, 2, 1),
            b3.reshape(NB, M3, 128).transpose(0, 2, 1),
        ],
        axis=2,
    ).astype(f)                                                  # [12, 128, 28]
    bqp = np.ascontiguousarray(bq[:, :, None], f)                # [12, 11, 1]
    return W1p, W2p, W3p, Wqp, bp, bqp


def kernel(x, W1, b1, W2, b2, W3, b3, Wv, bv, Wa, ba):
    global LAST_RESULT
    from concourse.bass_utils import run_bass_kernel_spmd

    x = np.asarray(x, np.float32)
    args = [np.asarray(a, np.float32) for a in (W1, b1, W2, b2, W3, b3, Wv, bv, Wa, ba)]
    W1p, W2p, W3p, Wqp, bp, bqp = _pack_weights(*args)

    nc = _build_nc()
    in_maps = []
    for c in range(NCORES):
        xT = np.ascontiguousarray(x[c * LB:(c + 1) * LB].T).astype(BF16)  # [249, 1024]
        in_maps.append({
            "xT": xT,
            "W1p": W1p, "W2p": W2p, "W3p": W3p, "Wqp": Wqp,
            "bp": bp, "bqp": bqp,
        })

    res = run_bass_kernel_spmd(nc, in_maps, list(range(NCORES)))
    LAST_RESULT = res

    out = np.empty((NB, B, NA), np.float32)
    for c in range(NCORES):
        out[:, c * LB:(c + 1) * LB, :] = res.results[c]["out"].transpose(0, 2, 1)
    return out


# revision 23
# speedup vs baseline: 1.0157x; 1.0057x over previous
"""Trainium2 Bass kernel for nn_BranchingQNetwork (12-branch dueling Q-MLP).

Strategy: data-parallel over batch (8 cores x 1024 rows). Per core, all 12
branch MLPs run as feature-major GEMM chains (weights stationary, activations
streaming) in bf16, k-outer single-pass accumulation in PSUM banks with
weights streamed through a small SBUF window. Layer-1 is zero-padded to
K=128 (pad rows of the px tile carry arbitrary finite x data times zero
weights) so every matmul has an identical 128-row shape and the PE never
reconfigures. Relu drains are whole-tile ops alternating between the scalar
and vector engines, emitted inline right after each accumulation group stops
so PSUM banks recycle early. The dueling head (v + a - mean(a)) is linear
and folded into a single [512, 11] weight on the host; it runs with Wq
stationary (11-column LDWEIGHTS, N=512 streaming) and the [11, batch] output
is transposed on the host. DMA queues: w2 on sync (prefetched 4 tiles ahead
into the next iteration), w3 + input/branch loads + output on gpsimd.
"""
import sys

sys.path.insert(0, "/opt/trn_rl_repo")

import numpy as np
import ml_dtypes

# problem dims (hardcoded per harness contract)
B = 8192
OBS = 249
NB = 12
NA = 11
NODE = 45
GRP = 17
D0 = 62
D1 = 2048
D2 = 1024
D3 = 512

NCORES = 8
LB = B // NCORES     # local batch per core
BT = 512             # batch tile
NBT = LB // BT
M1 = D1 // 128       # 16 output tiles of layer 1
K2 = D1 // 128       # 16 contraction tiles of layer 2
M2 = D2 // 128       # 8
K3 = D2 // 128       # 8
M3 = D3 // 128       # 4
KH = D3 // 128       # 4
PREF = 4             # w2 tiles prefetched during previous iteration

BF16 = ml_dtypes.bfloat16

_NC_CACHE = {}
LAST_RESULT = None


def _build_nc():
    if "nc" in _NC_CACHE:
        return _NC_CACHE["nc"]
    from concourse import bacc
    import concourse.mybir as mybir
    import concourse.tile as tile

    f32 = mybir.dt.float32
    bf16 = mybir.dt.bfloat16
    Relu = mybir.ActivationFunctionType.Relu
    Identity = mybir.ActivationFunctionType.Identity
    ADD = mybir.AluOpType.add
    MAX = mybir.AluOpType.max

    nc = bacc.Bacc("TRN2")

    xT_d = nc.declare_dram_parameter("xT", [OBS, LB], bf16, isOutput=False)
    W1_d = nc.declare_dram_parameter("W1p", [NB, 128, D1], bf16, isOutput=False)
    W2_d = nc.declare_dram_parameter("W2p", [NB, K2, 128, D2], bf16, isOutput=False)
    W3_d = nc.declare_dram_parameter("W3p", [NB, K3, 128, D3], bf16, isOutput=False)
    Wq_d = nc.declare_dram_parameter("Wqp", [NB, KH, 128, 128], bf16, isOutput=False)
    b_d = nc.declare_dram_parameter("bp", [NB, 128, M1 + M2 + M3], f32, isOutput=False)
    bq_d = nc.declare_dram_parameter("bqp", [NB, NA, 1], f32, isOutput=False)
    out_d = nc.declare_dram_parameter("out", [NB, NA, LB], f32, isOutput=True)

    with tile.TileContext(nc) as tc:
        with (
            tc.tile_pool(name="wp1", bufs=2) as wp1,
            tc.tile_pool(name="wp2", bufs=10) as wp2,
            tc.tile_pool(name="wp3", bufs=8) as wp3,
            tc.tile_pool(name="wpq", bufs=2) as wpq,
            tc.tile_pool(name="bbp", bufs=2) as bbp,
            tc.tile_pool(name="pxp", bufs=2) as pxp,
            tc.tile_pool(name="actp", bufs=1) as actp,
            tc.tile_pool(name="osp", bufs=2) as osp,
            tc.tile_pool(name="psp", bufs=8, space="PSUM") as psp,
        ):
            h1 = actp.tile([128, K2, BT], bf16, tag="h1")
            h2 = actp.tile([128, K3, BT], bf16, tag="h2")
            h3 = actp.tile([128, KH, BT], bf16, tag="h3")

            iters = [(br, bt) for br in range(NB) for bt in range(NBT)]
            loaded = {}
            pxs = {}
            w2ts = {}
            w2_issued = {}

            def load_branch(br, eng):
                w1t = wp1.tile([128, D1], bf16, tag="w1", name=f"w1_{br}")
                eng.dma_start(w1t[:], W1_d[br])
                wqt = wpq.tile([128, KH, 128], bf16, tag="wq", name=f"wq_{br}")
                eng.dma_start(wqt[:], Wq_d[br].rearrange("k p a -> p k a"))
                btile = bbp.tile([128, M1 + M2 + M3], f32, tag="b", name=f"b_{br}")
                eng.dma_start(btile[:], b_d[br])
                bqt = bbp.tile([NA, 1], f32, tag="bq", name=f"bq_{br}")
                eng.dma_start(bqt[:], bq_d[br])
                loaded[br] = (w1t, wqt, btile, bqt)

            def load_px(idx, eng):
                br, bt = iters[idx]
                bsl = slice(bt * BT, (bt + 1) * BT)
                g0 = NODE + GRP * br
                px = pxp.tile([128, BT], bf16, tag="px", name=f"px_{idx}")
                eng.dma_start(px[0:NODE, :], xT_d[0:NODE, bsl])
                eng.dma_start(px[NODE:D0, :], xT_d[g0:g0 + GRP, bsl])
                # rows 62-127 multiply zero weight rows; fill with finite x
                # data (never NaN) instead of memset to stay on one queue
                eng.dma_start(px[D0:128, :], xT_d[0:128 - D0, bsl])
                pxs[idx] = px

            def make_w2(idx):
                if idx in w2ts:
                    return
                w2ts[idx] = [
                    wp2.tile([128, D2], bf16, tag="w2", name=f"w2_{idx}_{k}")
                    for k in range(K2)
                ]
                w2_issued[idx] = set()

            def issue_w2(idx, k):
                if k >= K2 or k in w2_issued[idx]:
                    return
                w2_issued[idx].add(k)
                br = iters[idx][0]
                nc.sync.dma_start(w2ts[idx][k][:], W2_d[br, k])

            def drain(dst, ps, bias, j):
                if j % 2 == 0:
                    nc.scalar.activation(dst, ps, Relu, bias=bias, scale=1.0)
                else:
                    nc.vector.tensor_scalar(dst, ps, bias, 0.0, ADD, MAX)

            def emit_L1_mm(idx, m):
                br, _ = iters[idx]
                w1t, _, btile, _ = loaded[br]
                ps = psp.tile([128, BT], f32, tag="ps", name=f"l1ps_{idx}_{m}")
                nc.tensor.matmul(
                    ps[:], w1t[:, m * 128:(m + 1) * 128], pxs[idx][:],
                    start=True, stop=True,
                )
                drain(h1[:, m, :], ps[:], btile[:, m:m + 1], m)

            # prologue: warm the PE (HAM un-throttle needs ~3.4us of busy)
            # with dummy matmuls on a zeroed tile while the first DMAs land
            wrm = actp.tile([128, BT], bf16, tag="wrm")
            nc.gpsimd.memset(wrm[:], 0.0)
            for i in range(8):
                pw = psp.tile([128, BT], f32, tag="ps", name=f"warm_{i}")
                nc.tensor.matmul(pw[:], wrm[:, 0:128], wrm[:],
                                 start=True, stop=True)
            # layer-1 critical path (w1, px, biases) ahead of the w2 stream
            w1t0 = wp1.tile([128, D1], bf16, tag="w1", name="w1_0")
            nc.sync.dma_start(w1t0[:], W1_d[0])
            load_px(0, nc.sync)
            btile0 = bbp.tile([128, M1 + M2 + M3], f32, tag="b", name="b_0")
            nc.sync.dma_start(btile0[:], b_d[0])
            make_w2(0)
            issue_w2(0, 0)
            issue_w2(0, 1)
            wqt0 = wpq.tile([128, KH, 128], bf16, tag="wq", name="wq_0")
            nc.sync.dma_start(wqt0[:], Wq_d[0].rearrange("k p a -> p k a"))
            bqt0 = bbp.tile([NA, 1], f32, tag="bq", name="bq_0")
            nc.sync.dma_start(bqt0[:], bq_d[0])
            loaded[0] = (w1t0, wqt0, btile0, bqt0)
            for m in range(M1):
                emit_L1_mm(0, m)

            for idx, (br, bt) in enumerate(iters):
                w1t, wqt, btile, bqt = loaded[br]
                nxt = idx + 1 if idx + 1 < len(iters) else None

                # this iteration's w3 tiles stream on the gpsimd queue
                w3list = []
                for k in range(K3):
                    w3t = wp3.tile([128, D3], bf16, tag="w3", name=f"w3_{idx}_{k}")
                    nc.gpsimd.dma_start(w3t[:], W3_d[br, k])
                    w3list.append(w3t)
                if nxt is not None:
                    nbr = iters[nxt][0]
                    if nbr not in loaded:
                        load_branch(nbr, nc.gpsimd)
                    load_px(nxt, nc.gpsimd)
                    make_w2(nxt)

                # ---- L2: [2048 -> 1024], k-outer, 8 psum banks ----
                ps2 = [psp.tile([128, BT], f32, tag="ps", name=f"ps2_{idx}_{m}")
                       for m in range(M2)]
                for k in range(K2):
                    issue_w2(idx, k)
                    issue_w2(idx, k + 2)
                    w2t = w2ts[idx][k]
                    for m in range(M2):
                        nc.tensor.matmul(
                            ps2[m][:], w2t[:, m * 128:(m + 1) * 128],
                            h1[:, k, :],
                            start=(k == 0), stop=(k == K2 - 1),
                        )
                        if k == K2 - 1:
                            drain(h2[:, m, :], ps2[m][:],
                                  btile[:, M1 + m:M1 + m + 1], m)

                # ---- L3 [1024 -> 512] interleaved with next iteration's L1 ----
                ps3 = [psp.tile([128, BT], f32, tag="ps", name=f"ps3_{idx}_{m}")
                       for m in range(M3)]
                for k in range(K3):
                    for m in range(M3):
                        nc.tensor.matmul(
                            ps3[m][:], w3list[k][:, m * 128:(m + 1) * 128],
                            h2[:, k, :],
                            start=(k == 0), stop=(k == K3 - 1),
                        )
                        if k == K3 - 1:
                            drain(h3[:, m, :], ps3[m][:],
                                  btile[:, M1 + M2 + m:M1 + M2 + m + 1], m)
                    if nxt is not None:
                        emit_L1_mm(nxt, 2 * k)
                        emit_L1_mm(nxt, 2 * k + 1)
                        if k < PREF:
                            issue_w2(nxt, k)

                # ---- head: q = h3 @ Wq + bq, Wq stationary, [11, BT] out.
                # Two psum banks alternate so consecutive head matmuls never
                # hit the same bank (same-bank accumulation stalls ~95ns).
                # Ring padded to 32 allocs/iter for a stable rotation, with
                # psh_a/psh_b last so only the late ps2[6]/ps2[7] of the next
                # iteration inherit their banks.
                bsl = slice(bt * BT, (bt + 1) * BT)
                for p in range(2):
                    psp.tile([128, BT], f32, tag="ps", name=f"pad_{idx}_{p}")
                psh_a = psp.tile([128, BT], f32, tag="ps", name=f"psha_{idx}")
                psh_b = psp.tile([128, BT], f32, tag="ps", name=f"pshb_{idx}")
                for k in range(KH):
                    ps = psh_a if k % 2 == 0 else psh_b
                    nc.tensor.matmul(
                        ps[:], wqt[:, k, :], h3[:, k, :],
                        start=(k < 2), stop=(k >= 2),
                    )
                ost = osp.tile([NA, BT], f32, tag="os", name=f"ost_{idx}")
                nc.scalar.activation(ost[:], psh_a[0:NA, :], Identity,
                                     bias=bqt[:], scale=1.0)
                nc.vector.tensor_add(ost[:], ost[:], psh_b[0:NA, :])
                nc.gpsimd.dma_start(out_d[br, :, bsl], ost[:])

    nc.compile()
    _NC_CACHE["nc"] = nc
    return nc


def _pack_weights(W1, b1, W2, b2, W3, b3, Wv, bv, Wa, ba):
    f = np.float32
    W1p = np.zeros((NB, 128, D1), np.float32)
    W1p[:, :D0, :] = W1
    W1p = W1p.astype(BF16)                                       # [12, 128, 2048]
    W2p = np.ascontiguousarray(W2.reshape(NB, K2, 128, D2)).astype(BF16)
    W3p = np.ascontiguousarray(W3.reshape(NB, K3, 128, D3)).astype(BF16)
    # fold dueling head: q = h @ (Wv + Wa - mean(Wa)) + (bv + ba - mean(ba))
    Wq = Wv + Wa - Wa.mean(axis=2, keepdims=True)                # [12, 512, 11]
    bq = bv + ba - ba.mean(axis=1, keepdims=True)                # [12, 11]
    Wqp = np.zeros((NB, KH, 128, 128), np.float32)
    Wqp[:, :, :, :NA] = Wq.reshape(NB, KH, 128, NA)
    Wqp = Wqp.astype(BF16)
    bp = np.concatenate(
        [
            b1.reshape(NB, M1, 128).transpose(0, 2, 1),
            b2.reshape(NB, M2, 128).transpose(0, 2, 1),
            b3.reshape(NB, M3, 128).transpose(0, 2, 1),
        ],
        axis=2,
    ).astype(f)                                                  # [12, 128, 28]
    bqp = np.ascontiguousarray(bq[:, :, None], f)                # [12, 11, 1]
    return W1p, W2p, W3p, Wqp, bp, bqp


def kernel(x, W1, b1, W2, b2, W3, b3, Wv, bv, Wa, ba):
    global LAST_RESULT
    from concourse.bass_utils import run_bass_kernel_spmd

    x = np.asarray(x, np.float32)
    args = [np.asarray(a, np.float32) for a in (W1, b1, W2, b2, W3, b3, Wv, bv, Wa, ba)]
    W1p, W2p, W3p, Wqp, bp, bqp = _pack_weights(*args)

    nc = _build_nc()
    in_maps = []
    for c in range(NCORES):
        xT = np.ascontiguousarray(x[c * LB:(c + 1) * LB].T).astype(BF16)  # [249, 1024]
        in_maps.append({
            "xT": xT,
            "W1p": W1p, "W2p": W2p, "W3p": W3p, "Wqp": Wqp,
            "bp": bp, "bqp": bqp,
        })

    res = run_bass_kernel_spmd(nc, in_maps, list(range(NCORES)))
    LAST_RESULT = res

    out = np.empty((NB, B, NA), np.float32)
    for c in range(NCORES):
        out[:, c * LB:(c + 1) * LB, :] = res.results[c]["out"].transpose(0, 2, 1)
    return out


# revision 25
# speedup vs baseline: 1.0169x; 1.0011x over previous
"""Trainium2 Bass kernel for nn_BranchingQNetwork (12-branch dueling Q-MLP).

Strategy: data-parallel over batch (8 cores x 1024 rows). Per core, all 12
branch MLPs run as feature-major GEMM chains (weights stationary, activations
streaming) in bf16, k-outer single-pass accumulation in PSUM banks with
weights streamed through a small SBUF window. Layer-1 is zero-padded to
K=128 (pad rows of the px tile carry arbitrary finite x data times zero
weights) so every matmul has an identical 128-row shape and the PE never
reconfigures. Relu drains are whole-tile ops alternating between the scalar
and vector engines, emitted inline right after each accumulation group stops
so PSUM banks recycle early. The dueling head (v + a - mean(a)) is linear
and folded into a single [512, 11] weight on the host; it runs with Wq
stationary (11-column LDWEIGHTS, N=512 streaming) and the [11, batch] output
is transposed on the host. DMA queues: w2 on sync (prefetched 4 tiles ahead
into the next iteration), w3 + input/branch loads + output on gpsimd.
"""
import sys

sys.path.insert(0, "/opt/trn_rl_repo")

import numpy as np
import ml_dtypes

# problem dims (hardcoded per harness contract)
B = 8192
OBS = 249
NB = 12
NA = 11
NODE = 45
GRP = 17
D0 = 62
D1 = 2048
D2 = 1024
D3 = 512

NCORES = 8
LB = B // NCORES     # local batch per core
BT = 512             # batch tile
NBT = LB // BT
M1 = D1 // 128       # 16 output tiles of layer 1
K2 = D1 // 128       # 16 contraction tiles of layer 2
M2 = D2 // 128       # 8
K3 = D2 // 128       # 8
M3 = D3 // 128       # 4
KH = D3 // 128       # 4
PREF = 4             # w2 tiles prefetched during previous iteration

BF16 = ml_dtypes.bfloat16

_NC_CACHE = {}
LAST_RESULT = None


def _build_nc():
    if "nc" in _NC_CACHE:
        return _NC_CACHE["nc"]
    from concourse import bacc
    import concourse.mybir as mybir
    import concourse.tile as tile

    f32 = mybir.dt.float32
    bf16 = mybir.dt.bfloat16
    Relu = mybir.ActivationFunctionType.Relu
    Identity = mybir.ActivationFunctionType.Identity
    ADD = mybir.AluOpType.add
    MAX = mybir.AluOpType.max

    nc = bacc.Bacc("TRN2")

    xT_d = nc.declare_dram_parameter("xT", [OBS, LB], bf16, isOutput=False)
    W1_d = nc.declare_dram_parameter("W1p", [NB, 128, D1], bf16, isOutput=False)
    W2_d = nc.declare_dram_parameter("W2p", [NB, K2, 128, D2], bf16, isOutput=False)
    W3_d = nc.declare_dram_parameter("W3p", [NB, K3, 128, D3], bf16, isOutput=False)
    Wq_d = nc.declare_dram_parameter("Wqp", [NB, KH, 128, 128], bf16, isOutput=False)
    b_d = nc.declare_dram_parameter("bp", [NB, 128, M1 + M2 + M3], f32, isOutput=False)
    bq_d = nc.declare_dram_parameter("bqp", [NB, NA, 1], f32, isOutput=False)
    out_d = nc.declare_dram_parameter("out", [NB, NA, LB], f32, isOutput=True)

    with tile.TileContext(nc) as tc:
        with (
            tc.tile_pool(name="wp1", bufs=2) as wp1,
            tc.tile_pool(name="wp2", bufs=10) as wp2,
            tc.tile_pool(name="wp3", bufs=8) as wp3,
            tc.tile_pool(name="wpq", bufs=2) as wpq,
            tc.tile_pool(name="bbp", bufs=2) as bbp,
            tc.tile_pool(name="pxp", bufs=2) as pxp,
            tc.tile_pool(name="actp", bufs=1) as actp,
            tc.tile_pool(name="osp", bufs=2) as osp,
            tc.tile_pool(name="psp", bufs=8, space="PSUM") as psp,
        ):
            h1 = actp.tile([128, K2, BT], bf16, tag="h1")
            h2 = actp.tile([128, K3, BT], bf16, tag="h2")
            h3 = actp.tile([128, KH, BT], bf16, tag="h3")

            iters = [(br, bt) for br in range(NB) for bt in range(NBT)]
            loaded = {}
            pxs = {}
            w2ts = {}
            w2_issued = {}

            def load_branch(br, eng):
                w1t = wp1.tile([128, D1], bf16, tag="w1", name=f"w1_{br}")
                eng.dma_start(w1t[:], W1_d[br])
                wqt = wpq.tile([128, KH, 128], bf16, tag="wq", name=f"wq_{br}")
                eng.dma_start(wqt[:], Wq_d[br].rearrange("k p a -> p k a"))
                btile = bbp.tile([128, M1 + M2 + M3], f32, tag="b", name=f"b_{br}")
                eng.dma_start(btile[:], b_d[br])
                bqt = bbp.tile([NA, 1], f32, tag="bq", name=f"bq_{br}")
                eng.dma_start(bqt[:], bq_d[br])
                loaded[br] = (w1t, wqt, btile, bqt)

            def load_px(idx, eng):
                br, bt = iters[idx]
                bsl = slice(bt * BT, (bt + 1) * BT)
                g0 = NODE + GRP * br
                px = pxp.tile([128, BT], bf16, tag="px", name=f"px_{idx}")
                eng.dma_start(px[0:NODE, :], xT_d[0:NODE, bsl])
                eng.dma_start(px[NODE:D0, :], xT_d[g0:g0 + GRP, bsl])
                # rows 62-127 multiply zero weight rows; fill with finite x
                # data (never NaN) instead of memset to stay on one queue
                eng.dma_start(px[D0:128, :], xT_d[0:128 - D0, bsl])
                pxs[idx] = px

            def make_w2(idx):
                if idx in w2ts:
                    return
                w2ts[idx] = [
                    wp2.tile([128, D2], bf16, tag="w2", name=f"w2_{idx}_{k}")
                    for k in range(K2)
                ]
                w2_issued[idx] = set()

            def issue_w2(idx, k):
                if k >= K2 or k in w2_issued[idx]:
                    return
                w2_issued[idx].add(k)
                br = iters[idx][0]
                nc.sync.dma_start(w2ts[idx][k][:], W2_d[br, k])

            def drain(dst, ps, bias, j):
                if j % 2 == 0:
                    nc.scalar.activation(dst, ps, Relu, bias=bias, scale=1.0)
                else:
                    nc.vector.tensor_scalar(dst, ps, bias, 0.0, ADD, MAX)

            def emit_L1_mm(idx, m):
                br, _ = iters[idx]
                w1t, _, btile, _ = loaded[br]
                ps = psp.tile([128, BT], f32, tag="ps", name=f"l1ps_{idx}_{m}")
                nc.tensor.matmul(
                    ps[:], w1t[:, m * 128:(m + 1) * 128], pxs[idx][:],
                    start=True, stop=True,
                )
                drain(h1[:, m, :], ps[:], btile[:, m:m + 1], m)

            # prologue: warm the PE (HAM un-throttle needs ~3.4us of busy)
            # with dummy matmuls on a zeroed tile while the first DMAs land
            wrm = actp.tile([128, BT], bf16, tag="wrm")
            nc.gpsimd.memset(wrm[:], 0.0)
            for i in range(8):
                pw = psp.tile([128, BT], f32, tag="ps", name=f"warm_{i}")
                nc.tensor.matmul(pw[:], wrm[:, 0:128], wrm[:],
                                 start=True, stop=True)
            # layer-1 critical path split across three DMA queues: w1 on
            # sync, px on scalar, biases on gpsimd — all land in parallel
            w1t0 = wp1.tile([128, D1], bf16, tag="w1", name="w1_0")
            nc.sync.dma_start(w1t0[:], W1_d[0])
            load_px(0, nc.scalar)
            btile0 = bbp.tile([128, M1 + M2 + M3], f32, tag="b", name="b_0")
            nc.gpsimd.dma_start(btile0[:], b_d[0])
            bqt0 = bbp.tile([NA, 1], f32, tag="bq", name="bq_0")
            nc.gpsimd.dma_start(bqt0[:], bq_d[0])
            make_w2(0)
            issue_w2(0, 0)
            issue_w2(0, 1)
            wqt0 = wpq.tile([128, KH, 128], bf16, tag="wq", name="wq_0")
            nc.sync.dma_start(wqt0[:], Wq_d[0].rearrange("k p a -> p k a"))
            loaded[0] = (w1t0, wqt0, btile0, bqt0)
            for m in range(M1):
                emit_L1_mm(0, m)

            for idx, (br, bt) in enumerate(iters):
                w1t, wqt, btile, bqt = loaded[br]
                nxt = idx + 1 if idx + 1 < len(iters) else None

                # this iteration's w3 tiles stream on the gpsimd queue
                w3list = []
                for k in range(K3):
                    w3t = wp3.tile([128, D3], bf16, tag="w3", name=f"w3_{idx}_{k}")
                    nc.gpsimd.dma_start(w3t[:], W3_d[br, k])
                    w3list.append(w3t)
                if nxt is not None:
                    nbr = iters[nxt][0]
                    if nbr not in loaded:
                        load_branch(nbr, nc.gpsimd)
                    load_px(nxt, nc.gpsimd)
                    make_w2(nxt)

                # ---- L2: [2048 -> 1024], k-outer, 8 psum banks ----
                ps2 = [psp.tile([128, BT], f32, tag="ps", name=f"ps2_{idx}_{m}")
                       for m in range(M2)]
                for k in range(K2):
                    issue_w2(idx, k)
                    issue_w2(idx, k + 2)
                    w2t = w2ts[idx][k]
                    for m in range(M2):
                        nc.tensor.matmul(
                            ps2[m][:], w2t[:, m * 128:(m + 1) * 128],
                            h1[:, k, :],
                            start=(k == 0), stop=(k == K2 - 1),
                        )
                        if k == K2 - 1:
                            drain(h2[:, m, :], ps2[m][:],
                                  btile[:, M1 + m:M1 + m + 1], m)

                # ---- L3 [1024 -> 512] interleaved with next iteration's L1 ----
                ps3 = [psp.tile([128, BT], f32, tag="ps", name=f"ps3_{idx}_{m}")
                       for m in range(M3)]
                for k in range(K3):
                    # in the second-to-last block the L1 pair goes first so
                    # its drains clear the engines before block 7's h3
                    # drains, which the head matmuls wait on
                    if k == K3 - 2 and nxt is not None:
                        emit_L1_mm(nxt, 2 * k)
                        emit_L1_mm(nxt, 2 * k + 1)
                    for m in range(M3):
                        nc.tensor.matmul(
                            ps3[m][:], w3list[k][:, m * 128:(m + 1) * 128],
                            h2[:, k, :],
                            start=(k == 0), stop=(k == K3 - 1),
                        )
                        if k == K3 - 1:
                            drain(h3[:, m, :], ps3[m][:],
                                  btile[:, M1 + M2 + m:M1 + M2 + m + 1], m)
                    if k != K3 - 2 and nxt is not None:
                        emit_L1_mm(nxt, 2 * k)
                        emit_L1_mm(nxt, 2 * k + 1)
                    if nxt is not None and k < PREF:
                        issue_w2(nxt, k)

                # ---- head: q = h3 @ Wq + bq, Wq stationary, [11, BT] out.
                # Two psum banks alternate so consecutive head matmuls never
                # hit the same bank (same-bank accumulation stalls ~95ns).
                # Ring padded to 32 allocs/iter for a stable rotation, with
                # psh_a/psh_b last so only the late ps2[6]/ps2[7] of the next
                # iteration inherit their banks.
                bsl = slice(bt * BT, (bt + 1) * BT)
                for p in range(2):
                    psp.tile([128, BT], f32, tag="ps", name=f"pad_{idx}_{p}")
                psh_a = psp.tile([128, BT], f32, tag="ps", name=f"psha_{idx}")
                psh_b = psp.tile([128, BT], f32, tag="ps", name=f"pshb_{idx}")
                for k in range(KH):
                    ps = psh_a if k % 2 == 0 else psh_b
                    nc.tensor.matmul(
                        ps[:], wqt[:, k, :], h3[:, k, :],
                        start=(k < 2), stop=(k >= 2),
                    )
                ost = osp.tile([NA, BT], f32, tag="os", name=f"ost_{idx}")
                nc.scalar.activation(ost[:], psh_a[0:NA, :], Identity,
                                     bias=bqt[:], scale=1.0)
                nc.vector.tensor_add(ost[:], ost[:], psh_b[0:NA, :])
                nc.gpsimd.dma_start(out_d[br, :, bsl], ost[:])

    nc.compile()
    _NC_CACHE["nc"] = nc
    return nc


def _pack_weights(W1, b1, W2, b2, W3, b3, Wv, bv, Wa, ba):
    f = np.float32
    W1p = np.zeros((NB, 128, D1), np.float32)
    W1p[:, :D0, :] = W1
    W1p = W1p.astype(BF16)                                       # [12, 128, 2048]
    W2p = np.ascontiguousarray(W2.reshape(NB, K2, 128, D2)).astype(BF16)
    W3p = np.ascontiguousarray(W3.reshape(NB, K3, 128, D3)).astype(BF16)
    # fold dueling head: q = h @ (Wv + Wa - mean(Wa)) + (bv + ba - mean(ba))
    Wq = Wv + Wa - Wa.mean(axis=2, keepdims=True)                # [12, 512, 11]
    bq = bv + ba - ba.mean(axis=1, keepdims=True)                # [12, 11]
    Wqp = np.zeros((NB, KH, 128, 128), np.float32)
    Wqp[:, :, :, :NA] = Wq.reshape(NB, KH, 128, NA)
    Wqp = Wqp.astype(BF16)
    bp = np.concatenate(
        [
            b1.reshape(NB, M1, 128).transpose(0, 2, 1),
            b2.reshape(NB, M2, 128).transpose(0, 2, 1),
            b3.reshape(NB, M3, 128).transpose(0, 2, 1),
        ],
        axis=2,
    ).astype(f)                                                  # [12, 128, 28]
    bqp = np.ascontiguousarray(bq[:, :, None], f)                # [12, 11, 1]
    return W1p, W2p, W3p, Wqp, bp, bqp


def kernel(x, W1, b1, W2, b2, W3, b3, Wv, bv, Wa, ba):
    global LAST_RESULT
    from concourse.bass_utils import run_bass_kernel_spmd

    x = np.asarray(x, np.float32)
    args = [np.asarray(a, np.float32) for a in (W1, b1, W2, b2, W3, b3, Wv, bv, Wa, ba)]
    W1p, W2p, W3p, Wqp, bp, bqp = _pack_weights(*args)

    nc = _build_nc()
    in_maps = []
    for c in range(NCORES):
        xT = np.ascontiguousarray(x[c * LB:(c + 1) * LB].T).astype(BF16)  # [249, 1024]
        in_maps.append({
            "xT": xT,
            "W1p": W1p, "W2p": W2p, "W3p": W3p, "Wqp": Wqp,
            "bp": bp, "bqp": bqp,
        })

    res = run_bass_kernel_spmd(nc, in_maps, list(range(NCORES)))
    LAST_RESULT = res

    out = np.empty((NB, B, NA), np.float32)
    for c in range(NCORES):
        out[:, c * LB:(c + 1) * LB, :] = res.results[c]["out"].transpose(0, 2, 1)
    return out


# revision 28
# speedup vs baseline: 1.0196x; 1.0027x over previous
"""Trainium2 Bass kernel for nn_BranchingQNetwork (12-branch dueling Q-MLP).

Strategy: data-parallel over batch (8 cores x 1024 rows). Per core, all 12
branch MLPs run as feature-major GEMM chains (weights stationary, activations
streaming) in bf16, k-outer single-pass accumulation in PSUM banks with
weights streamed through a small SBUF window. Layer-1 is zero-padded to
K=128 (pad rows of the px tile carry arbitrary finite x data times zero
weights) so every matmul has an identical 128-row shape and the PE never
reconfigures. Relu drains are whole-tile ops alternating between the scalar
and vector engines, emitted inline right after each accumulation group stops
so PSUM banks recycle early. The dueling head (v + a - mean(a)) is linear
and folded into a single [512, 11] weight on the host; it runs with Wq
stationary (11-column LDWEIGHTS, N=512 streaming) and the [11, batch] output
is transposed on the host. DMA queues: w2 on sync (prefetched 4 tiles ahead
into the next iteration), w3 + input/branch loads + output on gpsimd.
"""
import sys

sys.path.insert(0, "/opt/trn_rl_repo")

import numpy as np
import ml_dtypes

# problem dims (hardcoded per harness contract)
B = 8192
OBS = 249
NB = 12
NA = 11
NODE = 45
GRP = 17
D0 = 62
D1 = 2048
D2 = 1024
D3 = 512

NCORES = 8
LB = B // NCORES     # local batch per core
BT = 512             # batch tile
NBT = LB // BT
M1 = D1 // 128       # 16 output tiles of layer 1
K2 = D1 // 128       # 16 contraction tiles of layer 2
M2 = D2 // 128       # 8
K3 = D2 // 128       # 8
M3 = D3 // 128       # 4
KH = D3 // 128       # 4
PREF = 4             # w2 tiles prefetched during previous iteration

BF16 = ml_dtypes.bfloat16

_NC_CACHE = {}
LAST_RESULT = None


def _build_nc():
    if "nc" in _NC_CACHE:
        return _NC_CACHE["nc"]
    from concourse import bacc
    import concourse.mybir as mybir
    import concourse.tile as tile

    f32 = mybir.dt.float32
    bf16 = mybir.dt.bfloat16
    Relu = mybir.ActivationFunctionType.Relu
    Identity = mybir.ActivationFunctionType.Identity
    ADD = mybir.AluOpType.add
    MAX = mybir.AluOpType.max

    nc = bacc.Bacc("TRN2")

    xT_d = nc.declare_dram_parameter("xT", [OBS, LB], bf16, isOutput=False)
    W1_d = nc.declare_dram_parameter("W1p", [NB, 128, D1], bf16, isOutput=False)
    W2_d = nc.declare_dram_parameter("W2p", [NB, K2, 128, D2], bf16, isOutput=False)
    W3_d = nc.declare_dram_parameter("W3p", [NB, K3, 128, D3], bf16, isOutput=False)
    Wq_d = nc.declare_dram_parameter("Wqp", [NB, KH, 128, 128], bf16, isOutput=False)
    b_d = nc.declare_dram_parameter("bp", [NB, 128, M1 + M2 + M3], f32, isOutput=False)
    bq_d = nc.declare_dram_parameter("bqp", [NB, NA, 1], f32, isOutput=False)
    out_d = nc.declare_dram_parameter("out", [NB, NA, LB], f32, isOutput=True)

    with tile.TileContext(nc) as tc:
        with (
            tc.tile_pool(name="wp1", bufs=2) as wp1,
            tc.tile_pool(name="wp2", bufs=10) as wp2,
            tc.tile_pool(name="wp3", bufs=8) as wp3,
            tc.tile_pool(name="wpq", bufs=2) as wpq,
            tc.tile_pool(name="bbp", bufs=2) as bbp,
            tc.tile_pool(name="pxp", bufs=2) as pxp,
            tc.tile_pool(name="actp", bufs=1) as actp,
            tc.tile_pool(name="osp", bufs=2) as osp,
            tc.tile_pool(name="psp", bufs=8, space="PSUM") as psp,
        ):
            h1 = actp.tile([128, K2, BT], bf16, tag="h1")
            h2 = actp.tile([128, K3, BT], bf16, tag="h2")
            h3 = actp.tile([128, KH, BT], bf16, tag="h3")

            iters = [(br, bt) for br in range(NB) for bt in range(NBT)]
            loaded = {}
            pxs = {}
            w2ts = {}
            w2_issued = {}

            def load_branch(br, eng):
                w1t = wp1.tile([128, D1], bf16, tag="w1", name=f"w1_{br}")
                eng.dma_start(w1t[:], W1_d[br])
                wqt = wpq.tile([128, KH, 128], bf16, tag="wq", name=f"wq_{br}")
                eng.dma_start(wqt[:], Wq_d[br].rearrange("k p a -> p k a"))
                btile = bbp.tile([128, M1 + M2 + M3], f32, tag="b", name=f"b_{br}")
                eng.dma_start(btile[:], b_d[br])
                bqt = bbp.tile([NA, 1], f32, tag="bq", name=f"bq_{br}")
                eng.dma_start(bqt[:], bq_d[br])
                loaded[br] = (w1t, wqt, btile, bqt)

            def load_px(idx, eng):
                br, bt = iters[idx]
                bsl = slice(bt * BT, (bt + 1) * BT)
                g0 = NODE + GRP * br
                px = pxp.tile([128, BT], bf16, tag="px", name=f"px_{idx}")
                eng.dma_start(px[0:NODE, :], xT_d[0:NODE, bsl])
                eng.dma_start(px[NODE:D0, :], xT_d[g0:g0 + GRP, bsl])
                # rows 62-127 multiply zero weight rows; fill with finite x
                # data (never NaN) instead of memset to stay on one queue
                eng.dma_start(px[D0:128, :], xT_d[0:128 - D0, bsl])
                pxs[idx] = px

            def make_w2(idx):
                if idx in w2ts:
                    return
                w2ts[idx] = [
                    wp2.tile([128, D2], bf16, tag="w2", name=f"w2_{idx}_{k}")
                    for k in range(K2)
                ]
                w2_issued[idx] = set()

            def issue_w2(idx, k):
                if k >= K2 or k in w2_issued[idx]:
                    return
                w2_issued[idx].add(k)
                br = iters[idx][0]
                nc.sync.dma_start(w2ts[idx][k][:], W2_d[br, k])

            def drain(dst, ps, bias, j):
                if j % 2 == 0:
                    nc.scalar.activation(dst, ps, Relu, bias=bias, scale=1.0)
                else:
                    nc.vector.tensor_scalar(dst, ps, bias, 0.0, ADD, MAX)

            def emit_L1_mm(idx, m):
                br, _ = iters[idx]
                w1t, _, btile, _ = loaded[br]
                ps = psp.tile([128, BT], f32, tag="ps", name=f"l1ps_{idx}_{m}")
                nc.tensor.matmul(
                    ps[:], w1t[:, m * 128:(m + 1) * 128], pxs[idx][:],
                    start=True, stop=True,
                )
                drain(h1[:, m, :], ps[:], btile[:, m:m + 1], m)

            # prologue: warm the PE (HAM un-throttle needs ~3.4us of busy)
            # with dummy matmuls on a zeroed tile while the first DMAs land
            wrm = actp.tile([128, BT], bf16, tag="wrm")
            nc.gpsimd.memset(wrm[:], 0.0)
            for i in range(8):
                pw = psp.tile([128, BT], f32, tag="ps", name=f"warm_{i}")
                nc.tensor.matmul(pw[:], wrm[:, 0:128], wrm[:],
                                 start=True, stop=True)
            # layer-1 critical path split across three DMA queues: w1 on
            # px then the first w1 half on sync (the first matmul only needs
            # w1[:, 0:128]); biases ride on gpsimd
            load_px(0, nc.sync)
            w1t0 = wp1.tile([128, D1], bf16, tag="w1", name="w1_0")
            nc.sync.dma_start(w1t0[:, 0:D1 // 2], W1_d[0][:, 0:D1 // 2])
            nc.sync.dma_start(w1t0[:, D1 // 2:D1], W1_d[0][:, D1 // 2:D1])
            btile0 = bbp.tile([128, M1 + M2 + M3], f32, tag="b", name="b_0")
            nc.gpsimd.dma_start(btile0[:], b_d[0])
            bqt0 = bbp.tile([NA, 1], f32, tag="bq", name="bq_0")
            nc.gpsimd.dma_start(bqt0[:], bq_d[0])
            make_w2(0)
            issue_w2(0, 0)
            issue_w2(0, 1)
            wqt0 = wpq.tile([128, KH, 128], bf16, tag="wq", name="wq_0")
            nc.sync.dma_start(wqt0[:], Wq_d[0].rearrange("k p a -> p k a"))
            loaded[0] = (w1t0, wqt0, btile0, bqt0)
            for m in range(M1):
                emit_L1_mm(0, m)

            for idx, (br, bt) in enumerate(iters):
                w1t, wqt, btile, bqt = loaded[br]
                nxt = idx + 1 if idx + 1 < len(iters) else None

                # this iteration's w3 tiles stream on the gpsimd queue
                w3list = []
                for k in range(K3):
                    w3t = wp3.tile([128, D3], bf16, tag="w3", name=f"w3_{idx}_{k}")
                    nc.gpsimd.dma_start(w3t[:], W3_d[br, k])
                    w3list.append(w3t)
                if nxt is not None:
                    nbr = iters[nxt][0]
                    if nbr not in loaded:
                        load_branch(nbr, nc.gpsimd)
                    load_px(nxt, nc.gpsimd)
                    make_w2(nxt)

                # ---- L2: [2048 -> 1024], k-outer, 8 psum banks ----
                ps2 = [psp.tile([128, BT], f32, tag="ps", name=f"ps2_{idx}_{m}")
                       for m in range(M2)]
                for k in range(K2):
                    issue_w2(idx, k)
                    issue_w2(idx, k + 2)
                    w2t = w2ts[idx][k]
                    for m in range(M2):
                        nc.tensor.matmul(
                            ps2[m][:], w2t[:, m * 128:(m + 1) * 128],
                            h1[:, k, :],
                            start=(k == 0), stop=(k == K2 - 1),
                        )
                        if k == K2 - 1:
                            drain(h2[:, m, :], ps2[m][:],
                                  btile[:, M1 + m:M1 + m + 1], m)

                # ---- L3 [1024 -> 512] interleaved with next iteration's L1 ----
                ps3 = [psp.tile([128, BT], f32, tag="ps", name=f"ps3_{idx}_{m}")
                       for m in range(M3)]
                for k in range(K3):
                    # in the second-to-last block the L1 pair goes first so
                    # its drains clear the engines before block 7's h3
                    # drains, which the head matmuls wait on
                    if k == K3 - 2 and nxt is not None:
                        emit_L1_mm(nxt, 2 * k)
                        emit_L1_mm(nxt, 2 * k + 1)
                    for m in range(M3):
                        nc.tensor.matmul(
                            ps3[m][:], w3list[k][:, m * 128:(m + 1) * 128],
                            h2[:, k, :],
                            start=(k == 0), stop=(k == K3 - 1),
                        )
                        if k == K3 - 1:
                            b3s = btile[:, M1 + M2 + m:M1 + M2 + m + 1]
                            if nxt is None:
                                # epilogue: split across both engines so the
                                # final head starts sooner
                                half = BT // 2
                                nc.scalar.activation(
                                    h3[:, m, 0:half], ps3[m][:, 0:half],
                                    Relu, bias=b3s, scale=1.0)
                                nc.vector.tensor_scalar(
                                    h3[:, m, half:BT], ps3[m][:, half:BT],
                                    b3s, 0.0, ADD, MAX)
                            else:
                                drain(h3[:, m, :], ps3[m][:], b3s, m)
                    if k != K3 - 2 and nxt is not None:
                        emit_L1_mm(nxt, 2 * k)
                        emit_L1_mm(nxt, 2 * k + 1)
                    if nxt is not None and k < PREF:
                        issue_w2(nxt, k)

                # ---- head: q = h3 @ Wq + bq, Wq stationary, [11, BT] out.
                # Two psum banks alternate so consecutive head matmuls never
                # hit the same bank (same-bank accumulation stalls ~95ns).
                # Ring padded to 32 allocs/iter for a stable rotation, with
                # psh_a/psh_b last so only the late ps2[6]/ps2[7] of the next
                # iteration inherit their banks.
                bsl = slice(bt * BT, (bt + 1) * BT)
                for p in range(2):
                    psp.tile([128, BT], f32, tag="ps", name=f"pad_{idx}_{p}")
                psh_a = psp.tile([128, BT], f32, tag="ps", name=f"psha_{idx}")
                psh_b = psp.tile([128, BT], f32, tag="ps", name=f"pshb_{idx}")
                for k in range(KH):
                    ps = psh_a if k % 2 == 0 else psh_b
                    nc.tensor.matmul(
                        ps[:], wqt[:, k, :], h3[:, k, :],
                        start=(k < 2), stop=(k >= 2),
                    )
                ost = osp.tile([NA, BT], f32, tag="os", name=f"ost_{idx}")
                if nxt is None:
                    # epilogue: halve the serial drain chain and flush the
                    # final output over two DMA queues
                    half = BT // 2
                    nc.scalar.activation(ost[:, 0:half], psh_a[0:NA, 0:half],
                                         Identity, bias=bqt[:], scale=1.0)
                    nc.vector.tensor_scalar(ost[:, half:BT],
                                            psh_a[0:NA, half:BT],
                                            bqt[:], None, ADD)
                    nc.vector.tensor_add(ost[:, 0:half], ost[:, 0:half],
                                         psh_b[0:NA, 0:half])
                    nc.vector.tensor_add(ost[:, half:BT], ost[:, half:BT],
                                         psh_b[0:NA, half:BT])
                    h0 = bt * BT
                    nc.gpsimd.dma_start(out_d[br, :, h0:h0 + half],
                                        ost[:, 0:half])
                    nc.sync.dma_start(out_d[br, :, h0 + half:h0 + BT],
                                      ost[:, half:BT])
                else:
                    nc.scalar.activation(ost[:], psh_a[0:NA, :], Identity,
                                         bias=bqt[:], scale=1.0)
                    nc.vector.tensor_add(ost[:], ost[:], psh_b[0:NA, :])
                    nc.gpsimd.dma_start(out_d[br, :, bsl], ost[:])

    nc.compile()
    _NC_CACHE["nc"] = nc
    return nc


def _pack_weights(W1, b1, W2, b2, W3, b3, Wv, bv, Wa, ba):
    f = np.float32
    W1p = np.zeros((NB, 128, D1), np.float32)
    W1p[:, :D0, :] = W1
    W1p = W1p.astype(BF16)                                       # [12, 128, 2048]
    W2p = np.ascontiguousarray(W2.reshape(NB, K2, 128, D2)).astype(BF16)
    W3p = np.ascontiguousarray(W3.reshape(NB, K3, 128, D3)).astype(BF16)
    # fold dueling head: q = h @ (Wv + Wa - mean(Wa)) + (bv + ba - mean(ba))
    Wq = Wv + Wa - Wa.mean(axis=2, keepdims=True)                # [12, 512, 11]
    bq = bv + ba - ba.mean(axis=1, keepdims=True)                # [12, 11]
    Wqp = np.zeros((NB, KH, 128, 128), np.float32)
    Wqp[:, :, :, :NA] = Wq.reshape(NB, KH, 128, NA)
    Wqp = Wqp.astype(BF16)
    bp = np.concatenate(
        [
            b1.reshape(NB, M1, 128).transpose(0, 2, 1),
            b2.reshape(NB, M2, 128).transpose(0, 2, 1),
            b3.reshape(NB, M3, 128).transpose(0, 2, 1),
        ],
        axis=2,
    ).astype(f)                                                  # [12, 128, 28]
    bqp = np.ascontiguousarray(bq[:, :, None], f)                # [12, 11, 1]
    return W1p, W2p, W3p, Wqp, bp, bqp


def kernel(x, W1, b1, W2, b2, W3, b3, Wv, bv, Wa, ba):
    global LAST_RESULT
    from concourse.bass_utils import run_bass_kernel_spmd

    x = np.asarray(x, np.float32)
    args = [np.asarray(a, np.float32) for a in (W1, b1, W2, b2, W3, b3, Wv, bv, Wa, ba)]
    W1p, W2p, W3p, Wqp, bp, bqp = _pack_weights(*args)

    nc = _build_nc()
    in_maps = []
    for c in range(NCORES):
        xT = np.ascontiguousarray(x[c * LB:(c + 1) * LB].T).astype(BF16)  # [249, 1024]
        in_maps.append({
            "xT": xT,
            "W1p": W1p, "W2p": W2p, "W3p": W3p, "Wqp": Wqp,
            "bp": bp, "bqp": bqp,
        })

    res = run_bass_kernel_spmd(nc, in_maps, list(range(NCORES)))
    LAST_RESULT = res

    out = np.empty((NB, B, NA), np.float32)
    for c in range(NCORES):
        out[:, c * LB:(c + 1) * LB, :] = res.results[c]["out"].transpose(0, 2, 1)
    return out
